# revision 25
# baseline (speedup 1.0000x reference)
"""Trainium2 Bass kernel for nn_CDQNModel (dueling CDQN with residual MLP towers).

Sharding: data-parallel over batch B=16 across 8 NeuronCores (2 batches/core).
Per core the kernel runs the full model on its batch shard:
  - global tower (60->256->512->1024 res blocks + layernorm) over 4096 rows
  - max over s2 -> gf [1024, 64]
  - comb tower (1024->512->256->128) + head -> q_comb [64]
  - fine tower (1084->512->256->128) + head -> q_fine [128]
  - mask/pad combine -> q [2, 64]
Weights are replicated on every core. Activations keep features on SBUF
partitions (layout A); layernorm transposes row-tiles with the PE, computes
stats with bn_stats in fp32, and transposes back. Matmul operands are bf16
(fp32 PSUM accumulation); the LN statistics, bias/beta, and the final mask
combine stay fp32. block2->block3 activations bounce through DRAM to keep
block3's weights resident in SBUF. Emission is software-pipelined so LN
stats windows are covered by independent FC matmuls.
"""

import numpy as np
from contextlib import ExitStack

import jax
import concourse.mybir as mybir
import concourse.tile as tile
from concourse import bacc
from concourse import bass2jax
from jax.sharding import Mesh, PartitionSpec
from jax.experimental.shard_map import shard_map

F32 = mybir.dt.float32
F32R = mybir.dt.bfloat16  # matmul operand dtype (bf16: full PE rate at any N)
AF = mybir.ActivationFunctionType
ALU = mybir.AluOpType
AX = mybir.AxisListType

B, S1, S2, F = 16, 32, 64, 60
NC = 8
BPC = B // NC            # batches per core
R = BPC * S1 * S2        # 4096 rows per core
G12 = 512                # rows per group, blocks 1-2
G3 = 512                 # rows per group, block 3
NEG = 100000.0
LN_EPS = 1e-12

# (name, kc_in, u) per res block; kc_in counts 128-wide K chunks (padded)
BLOCKS = {
    "g1": (1, 256), "g2": (2, 512), "g3": (4, 1024),
    "c1": (8, 512), "c2": (4, 256), "c3": (2, 128),
    "f1": (9, 512), "f2": (4, 256), "f3": (2, 128),
}
# bias chunk offsets into the packed [5, 3584] bias tensor
_BOFS = {}
_o = 0
for _n in ["g1", "g2", "g3", "c1", "c2", "c3", "f1", "f2", "f3"]:
    _BOFS[_n] = _o
    _o += BLOCKS[_n][1] // 128
BIAS_CHUNKS = _o  # 28

_EXEC = None  # cached (fn, in_names, n_params, out_shapes)


def _res_fc(nc, tc, name, xks, G, pools, bias_t, wts):
    """FC part of a res block: r1, r2, y = relu-stack + shortcut sum."""
    kc, u = BLOCKS[name]
    mc = u // 128
    P = 128
    bofs = _BOFS[name]
    big, tmp_pool, stat_pool, psum_mm, psum_tr = pools

    def fc(src_ks, w_t, brow, out, alt0=0):
        n_k = len(src_ks)
        for m in range(mc):
            ps = psum_mm.tile([P, G], F32, tag="fc")
            for k in range(n_k):
                nc.tensor.matmul(ps[:, :G], lhsT=w_t[:, k, m * 128:(m + 1) * 128],
                                 rhs=src_ks[k], start=(k == 0), stop=(k == n_k - 1))
            b_ap = bias_t[:, brow, bofs + m:bofs + m + 1]
            if (m + alt0) % 2 == 0:
                nc.scalar.activation(out[:, m, :], ps[:, :G], AF.Relu, bias=b_ap)
            else:
                nc.vector.tensor_scalar(out[:, m, :], ps[:, :G], b_ap, 0.0,
                                        ALU.add, ALU.max)

    r1 = big.tile([P, mc, G], F32R, tag="bigA")
    fc(xks, wts["w1"], 0, r1)
    r1_ks = [r1[:, m, :] for m in range(mc)]
    r2 = big.tile([P, mc, G], F32R, tag="bigB")
    fc(r1_ks, wts["w2"], 1, r2, alt0=1)
    r2_ks = [r2[:, m, :] for m in range(mc)]

    y = big.tile([P, mc, G], F32R, tag="bigY")
    fc(xks, wts["ws"], 3, y)
    for m in range(mc):
        ps = psum_mm.tile([P, G], F32, tag="fc")
        for k in range(mc):
            nc.tensor.matmul(ps[:, :G], lhsT=wts["w3"][:, k, m * 128:(m + 1) * 128],
                             rhs=r2_ks[k], start=(k == 0), stop=(k == mc - 1))
        t = tmp_pool.tile([P, G], F32R, tag="tmp")
        b_ap = bias_t[:, 2, bofs + m:bofs + m + 1]
        if m % 2 == 0:
            nc.scalar.activation(t[:, :G], ps[:, :G], AF.Relu, bias=b_ap)
        else:
            nc.vector.tensor_scalar(t[:, :G], ps[:, :G], b_ap, 0.0, ALU.add, ALU.max)
        eng = nc.gpsimd if (mc >= 8 and m % 2 == 0) else nc.vector
        eng.tensor_tensor(y[:, m, :], y[:, m, :], t[:, :G], ALU.add)
    return y


def _res_ln_fwd(nc, tc, name, y, G, pools, rows=None):
    """Forward transpose y -> yB row-tiles, then stats + normalize (DVE)."""
    kc, u = BLOCKS[name]
    mc = u // 128
    P = 128
    big, tmp_pool, stat_pool, psum_mm, psum_tr = pools
    ident = tc._ident
    eps = tc._eps
    act_rows = G if rows is None else rows
    nrt = max(1, act_rows // P)
    rp = min(P, act_rows)
    yB = big.tile([P, nrt, u], F32R, tag="bigYB")
    for rt in range(nrt):
        n_half = (u + 511) // 512
        for h in range(n_half):
            mlo = h * 4
            mhi = min(mc, mlo + 4)
            pst = psum_tr.tile([P, 512], F32R, tag="tr")
            for m in range(mlo, mhi):
                nc.tensor.transpose(pst[:rp, (m - mlo) * 128:(m - mlo + 1) * 128],
                                    y[:, m, rt * rp:(rt + 1) * rp], ident)
            if (rt + h) % 2 == 0:
                nc.scalar.copy(yB[:rp, rt, mlo * 128:mhi * 128],
                               pst[:rp, :(mhi - mlo) * 128])
            else:
                nc.vector.tensor_copy(yB[:rp, rt, mlo * 128:mhi * 128],
                                      pst[:rp, :(mhi - mlo) * 128])
    for rt in range(nrt):
        nseg = max(1, u // 512)
        seg = u // nseg
        sm = stat_pool.tile([P, 16], F32, tag="statsmv")
        stats = sm[:, 0:12].rearrange("p (s x) -> p s x", x=6)
        mv = sm[:, 12:14]
        for s in range(nseg):
            nc.vector.bn_stats(stats[:rp, s, :], yB[:rp, rt, s * seg:(s + 1) * seg])
        nc.vector.bn_aggr(mv[:rp], stats[:rp, 0:nseg, :])
        nc.scalar.activation(mv[:rp, 1:2], mv[:rp, 1:2], AF.Sqrt, bias=eps[:rp])
        nc.vector.reciprocal(mv[:rp, 1:2], mv[:rp, 1:2])
        nc.vector.tensor_scalar(yB[:rp, rt, :], yB[:rp, rt, :], mv[:rp, 0:1],
                                mv[:rp, 1:2], ALU.subtract, ALU.mult)
    return yB


def _res_ln_back(nc, tc, name, yB, G, pools, bias_t, out_gf=None, out_pool=None,
                 rows=None):
    """Back transpose (+beta) -> yln F32R tiles, or fused s2-max into gf_raw."""
    kc, u = BLOCKS[name]
    mc = u // 128
    P = 128
    bofs = _BOFS[name]
    big, tmp_pool, stat_pool, psum_mm, psum_tr = pools
    ident = tc._ident
    act_rows = G if rows is None else rows
    nrt = max(1, act_rows // P)
    rp = min(P, act_rows)
    yln = None
    if out_gf is None:
        yln = (out_pool or big).tile([P, mc, G], F32R, tag=f"yln{name[0]}")
    for m in range(mc):
        pst = psum_tr.tile([P, 512], F32R, tag="tr")
        for rt in range(nrt):
            nc.tensor.transpose(pst[:, rt * rp:(rt + 1) * rp],
                                yB[:rp, rt, m * 128:(m + 1) * 128],
                                ident[:rp, :rp])
        if out_gf is None:
            b_ap = bias_t[:, 4, bofs + m:bofs + m + 1]
            if m % 2 == 0:
                nc.scalar.activation(yln[:, m, :act_rows], pst[:, :act_rows],
                                     AF.Identity, bias=b_ap)
            else:
                nc.vector.tensor_scalar(yln[:, m, :act_rows], pst[:, :act_rows],
                                        b_ap, None, ALU.add)
        else:
            gf_raw, cell0 = out_gf
            ncell = G // 64
            nc.vector.tensor_reduce(
                gf_raw[:, m, cell0:cell0 + ncell],
                pst[:, :G].rearrange("p (c s) -> p c s", s=64),
                axis=AX.X, op=ALU.max)
    return yln


def _build():
    nc = bacc.Bacc("TRN2", target_bir_lowering=False, debug=False)

    xt = nc.dram_tensor("xt", (128, R), F32R, kind="ExternalInput")
    wdr = {}
    for name, (kc, u) in BLOCKS.items():
        wdr[name] = {
            "w1": nc.dram_tensor(f"{name}_w1", (kc * 128, u), F32R, kind="ExternalInput"),
            "w2": nc.dram_tensor(f"{name}_w2", (u, u), F32R, kind="ExternalInput"),
            "w3": nc.dram_tensor(f"{name}_w3", (u, u), F32R, kind="ExternalInput"),
            "ws": nc.dram_tensor(f"{name}_ws", (kc * 128, u), F32R, kind="ExternalInput"),
        }
    bias_d = nc.dram_tensor("bias_all", (128, 5, BIAS_CHUNKS), F32, kind="ExternalInput")
    wv2_d = nc.dram_tensor("wv2", (128, 2), F32R, kind="ExternalInput")
    mk_d = nc.dram_tensor("mk", (1, 4, 128), F32, kind="ExternalInput")
    h2s = nc.dram_tensor("h2s", (4, 128, R), F32R, kind="Internal")
    out_d = nc.dram_tensor("out", (1, BPC * S2), F32, kind="ExternalOutput")

    NGRP = R // G12
    NG3 = R // G3

    with tile.TileContext(nc) as tc, ExitStack() as ctx:
        from concourse.masks import make_identity
        singles = ctx.enter_context(tc.tile_pool(name="singles", bufs=1))
        tmp_pool = ctx.enter_context(tc.tile_pool(name="tmp", bufs=2))
        stat_pool = ctx.enter_context(tc.tile_pool(name="stats", bufs=2))
        psum_mm = ctx.enter_context(tc.tile_pool(name="psum_mm", bufs=5, space="PSUM"))
        psum_tr = ctx.enter_context(tc.tile_pool(name="psum_tr", bufs=3, space="PSUM"))

        ident = singles.tile([128, 128], F32R)
        make_identity(nc, ident)
        tc._ident = ident
        eps = singles.tile([128, 1], F32)
        nc.vector.memset(eps, LN_EPS)
        tc._eps = eps

        bias_t = singles.tile([128, 5, BIAS_CHUNKS], F32)
        nc.sync.dma_start(bias_t, bias_d.ap())
        wv2_t = singles.tile([128, 2], F32R)
        nc.sync.dma_start(wv2_t, wv2_d.ap())
        mk_t = singles.tile([1, 4, 128], F32)
        nc.sync.dma_start(mk_t, mk_d.ap())
        gf_raw = singles.tile([128, 8, 64], F32)
        gf = singles.tile([128, 8, 64], F32R)

        # W3 + h2-read pools opened below the P12 pools (regions don't overlap
        # P12 pools, so their DMAs can run during blocks 1-2)
        mid = ctx.enter_context(ExitStack())
        w3p = mid.enter_context(tc.tile_pool(name="w3p", bufs=1))
        h2rd = mid.enter_context(tc.tile_pool(name="h2rd", bufs=3))

        # ---------------- blocks 1-2 ----------------
        with tc.tile_pool(name="xtp", bufs=1) as xtp, \
             tc.tile_pool(name="w12", bufs=1) as w12, \
             tc.tile_pool(name="big12", bufs=3) as big12, \
             tc.tile_pool(name="yln12", bufs=3) as yln12:
            xT = xtp.tile([128, R], F32R)
            for g in range(NGRP):
                eng = nc.sync if g % 2 == 0 else nc.scalar
                eng.dma_start(xT[:, g * G12:(g + 1) * G12],
                              xt.ap()[:, g * G12:(g + 1) * G12])

            wt12 = {}
            _i = 0
            for name in ("g1", "g2"):
                kc, u = BLOCKS[name]
                wt12[name] = {}
                for wn in ("w1", "w2", "w3", "ws"):
                    d = wdr[name][wn]
                    ckc = d.shape[0] // 128
                    t = w12.tile([128, ckc, u], F32R, tag=f"{name}_{wn}")
                    eng = nc.scalar if _i % 2 == 0 else nc.sync
                    _i += 1
                    eng.dma_start(t, d.ap().rearrange("(c p) f -> p c f", p=128))
                    wt12[name][wn] = t

            # W3 prefetch streams during blocks 1-2 (issued after the loads above)
            wt3 = {}
            for wn in ("w1", "w2", "w3", "ws"):
                d = wdr["g3"][wn]
                ckc = d.shape[0] // 128
                t = w3p.tile([128, ckc, 1024], F32R, tag=f"g3_{wn}")
                nc.gpsimd.dma_start(t, d.ap().rearrange("(c p) f -> p c f", p=128))
                wt3[wn] = t

            pools12 = (big12, tmp_pool, stat_pool, psum_mm, psum_tr)
            # software-pipelined emission: back-transposes are emitted after
            # independent FC work so the PE never waits on LN stats
            yB2_prev = None

            def back2_flush(gp):
                y2 = _res_ln_back(nc, tc, "g2", yB2_prev, G12, pools12, bias_t,
                                  out_pool=yln12)
                nc.sync.dma_start(
                    h2s.ap()[:, :, gp * G12:(gp + 1) * G12].rearrange("c p j -> p c j"),
                    y2)

            for g in range(NGRP):
                xg = [xT[:, g * G12:(g + 1) * G12]]
                y1 = _res_fc(nc, tc, "g1", xg, G12, pools12, bias_t, wt12["g1"])
                yB1 = _res_ln_fwd(nc, tc, "g1", y1, G12, pools12)
                if g > 0:
                    back2_flush(g - 1)
                yln1 = _res_ln_back(nc, tc, "g1", yB1, G12, pools12, bias_t,
                                    out_pool=yln12)
                y1_ks = [yln1[:, m, :] for m in range(2)]
                y2 = _res_fc(nc, tc, "g2", y1_ks, G12, pools12, bias_t, wt12["g2"])
                yB2_prev = _res_ln_fwd(nc, tc, "g2", y2, G12, pools12)
            back2_flush(NGRP - 1)

        # ---------------- block 3 (+ fused s2-max) ----------------
        with tc.tile_pool(name="big3", bufs=2) as big3:
            pools3 = (big3, tmp_pool, stat_pool, psum_mm, psum_tr)
            yB3_prev = None
            for g in range(NG3):
                h2g = h2rd.tile([128, 4, G3], F32R, tag="h2g")
                nc.scalar.dma_start(
                    h2g, h2s.ap()[:, :, g * G3:(g + 1) * G3].rearrange("c p j -> p c j"))
                xg = [h2g[:, k, :] for k in range(4)]
                y3 = _res_fc(nc, tc, "g3", xg, G3, pools3, bias_t, wt3)
                if g > 0:
                    _res_ln_back(nc, tc, "g3", yB3_prev, G3, pools3, bias_t,
                                 out_gf=(gf_raw, (g - 1) * (G3 // 64)))
                yB3_prev = _res_ln_fwd(nc, tc, "g3", y3, G3, pools3)
            _res_ln_back(nc, tc, "g3", yB3_prev, G3, pools3, bias_t,
                         out_gf=(gf_raw, (NG3 - 1) * (G3 // 64)))
            for m in range(8):
                nc.scalar.activation(gf[:, m, :], gf_raw[:, m, :], AF.Identity,
                                     bias=bias_t[:, 4, _BOFS["g3"] + m:_BOFS["g3"] + m + 1])

        mid.close()  # free W3 + h2rd regions for the tower pools

        # ---------------- comb + fine towers ----------------
        with tc.tile_pool(name="wq", bufs=2) as wq, \
             tc.tile_pool(name="bigq", bufs=2) as bigq:
            def load_w(name):
                kc, u = BLOCKS[name]
                out = {}
                for wn in ("w1", "w2", "w3", "ws"):
                    d = wdr[name][wn]
                    ckc = d.shape[0] // 128
                    # one tag per role sized to the max (9x512) so the pool
                    # holds 2 slots per role, not one per shape
                    tf = wq.tile([128, 9, 512], F32R, tag=f"qw_{wn}")
                    t = tf[:, :ckc, :u]
                    nc.gpsimd.dma_start(t, d.ap().rearrange("(c p) f -> p c f", p=128))
                    out[wn] = t
                return out

            poolsq = (bigq, tmp_pool, stat_pool, psum_mm, psum_tr)
            # fine tower input sfT [1152, 128]
            sfT = bigq.tile([128, 9, 128], F32R, tag="sfT")
            for m in range(8):
                for b in range(BPC):
                    nc.vector.tensor_copy(sfT[:, m, b * 64:(b + 1) * 64],
                                          gf[:, m, b * 32:b * 32 + 1].to_broadcast((128, 64)))
            for b in range(BPC):
                nc.sync.dma_start(sfT[:, 8, b * 64:(b + 1) * 64],
                                  xt.ap()[:, b * (S1 * S2):b * (S1 * S2) + S2])

            # interleave comb (rows=64) and fine (rows=128) towers: each tower's
            # LN stats window is covered by the other tower's FC matmuls
            state = {"c": (gf, 64), "f": (sfT, 128)}
            pending = {}  # tk -> (name, yB)
            for i in (1, 2, 3):
                for tk in ("c", "f"):
                    name = f"{tk}{i}"
                    w_t = load_w(name)
                    kc, u = BLOCKS[name]
                    xcur, rows = state[tk]
                    x_ks = [xcur[:, k, :] for k in range(kc)]
                    yt = _res_fc(nc, tc, name, x_ks, rows, poolsq, bias_t, w_t)
                    yBt = _res_ln_fwd(nc, tc, name, yt, rows, poolsq, rows=rows)
                    pending[tk] = (name, yBt)
                    # flush the OTHER tower's pending back-transpose
                    ok = "f" if tk == "c" else "c"
                    if ok in pending and (i > 1 or tk == "f"):
                        pname, pyB = pending.pop(ok)
                        prows = state[ok][1]
                        ylnp = _res_ln_back(nc, tc, pname, pyB, prows, poolsq,
                                            bias_t, rows=prows)
                        state[ok] = (ylnp, prows)
            for tk in ("c", "f"):
                if tk in pending:
                    pname, pyB = pending.pop(tk)
                    prows = state[tk][1]
                    ylnp = _res_ln_back(nc, tc, pname, pyB, prows, poolsq,
                                        bias_t, rows=prows)
                    state[tk] = (ylnp, prows)
            xc = state["c"][0]
            xf = state["f"][0]
            psh = psum_mm.tile([1, 64], F32, tag="fc")
            nc.tensor.matmul(psh, lhsT=wv2_t[:, 0:1], rhs=xc[:, 0, :], start=True, stop=True)
            qc = bigq.tile([1, 64], F32, tag="qc")
            nc.vector.tensor_copy(qc, psh)
            psf = psum_mm.tile([1, 128], F32, tag="fc")
            nc.tensor.matmul(psf, lhsT=wv2_t[:, 1:2], rhs=xf[:, 0, :], start=True, stop=True)
            qf = bigq.tile([1, 128], F32, tag="qf")
            nc.vector.tensor_copy(qf, psf)

            # combine: q = qf*cf + qadd ; q[:, :, :32] += qc*cmpad
            nc.vector.tensor_tensor(qc, qc, mk_t[:, 0, 0:64], ALU.mult)
            nc.vector.tensor_tensor(qf, qf, mk_t[:, 1, :], ALU.mult)
            q = bigq.tile([1, 128], F32, tag="q")
            nc.vector.tensor_tensor(q, qf, mk_t[:, 2, :], ALU.add)
            qv = q.rearrange("p (b j) -> p b j", j=64)
            qcv = qc.rearrange("p (b j) -> p b j", j=32)
            nc.vector.tensor_tensor(qv[:, :, 0:32], qv[:, :, 0:32], qcv, ALU.add)
            nc.sync.dma_start(out_d.ap(), q)

    nc.finalize()
    return nc


def _make_executor():
    """Build nc once and return a cached jitted shard_map executor
    (mirrors bass2jax.run_bass_via_pjrt's multi-core path)."""
    nc = _build()
    bass2jax.install_neuronx_cc_hook()

    in_names, out_names, out_avals, zero_shapes = [], [], [], []
    for alloc in nc.m.functions[0].allocations:
        if not isinstance(alloc, mybir.MemoryLocationSet):
            continue
        name = alloc.memorylocations[0].name
        if alloc.kind == "ExternalInput":
            in_names.append(name)
        elif alloc.kind == "ExternalOutput":
            out_names.append(name)
            shape = tuple(alloc.tensor_shape)
            dtype = mybir.dt.np(alloc.dtype)
            out_avals.append(jax.core.ShapedArray(shape, dtype))
            zero_shapes.append((shape, dtype))
    partition_name = nc.partition_id_tensor.name if nc.partition_id_tensor else None
    in_names = [n for n in in_names if n != partition_name]
    n_params = len(in_names)
    all_names = in_names + out_names
    if partition_name is not None:
        all_names = all_names + [partition_name]

    def _body(*args):
        operands = list(args)
        if partition_name is not None:
            operands.append(bass2jax.partition_id_tensor())
        outs = bass2jax._bass_exec_p.bind(
            *operands,
            out_avals=tuple(out_avals),
            in_names=tuple(all_names),
            out_names=tuple(out_names),
            lowering_input_output_aliases=(),
            sim_require_finite=True,
            sim_require_nnan=True,
            nc=nc,
        )
        return tuple(outs)

    devices = jax.devices()[:NC]
    mesh = Mesh(np.asarray(devices), ("core",))
    n_outs = len(out_names)
    sharded = jax.jit(
        shard_map(_body, mesh=mesh,
                  in_specs=(PartitionSpec("core"),) * (n_params + n_outs),
                  out_specs=(PartitionSpec("core"),) * n_outs,
                  check_rep=False),
        donate_argnums=tuple(range(n_params, n_params + n_outs)),
        keep_unused=True,
    )
    return sharded, in_names, out_names, zero_shapes


def _prep_inputs(joint_state, comb_mask, fine_mask, params):
    """Host-side: weights (replicated) + per-core shards, keyed by dram name."""
    import ml_dtypes
    BF = ml_dtypes.bfloat16

    def npf(a):
        return np.ascontiguousarray(np.asarray(a, np.float32))

    def npb(a):
        return np.ascontiguousarray(np.asarray(a, np.float32).astype(BF))

    w = {}
    towers = {"g": params["global"], "c": params["comb"], "f": params["fine"]}
    for tk, blocks in towers.items():
        for i, blk in enumerate(blocks):
            name = f"{tk}{i + 1}"
            kc, u = BLOCKS[name]
            w1, b1 = blk["stack"][0]
            w2, b2 = blk["stack"][1]
            w3, b3 = blk["stack"][2]
            ws, bs = blk["ws"], blk["bs"]
            beta = blk["beta"]
            d_in = npf(w1).shape[0]
            w1p = np.zeros((kc * 128, u), np.float32)
            w1p[:d_in] = npf(w1)
            wsp = np.zeros((kc * 128, u), np.float32)
            wsp[:d_in] = npf(ws)
            w[f"{name}_w1"] = w1p.astype(BF)
            w[f"{name}_w2"] = npb(w2)
            w[f"{name}_w3"] = npb(w3)
            w[f"{name}_ws"] = wsp.astype(BF)
            w[f"{name}_bias"] = np.stack([npf(b1), npf(b2), npf(b3), npf(bs), npf(beta)])

    bias_flat = np.zeros((5, BIAS_CHUNKS * 128), np.float32)
    for name in BLOCKS:
        u = BLOCKS[name][1]
        o = _BOFS[name] * 128
        bias_flat[:, o:o + u] = w.pop(f"{name}_bias")
    # [5, c*128+p] -> [p, 5, c]
    bias_all = np.ascontiguousarray(
        bias_flat.reshape(5, BIAS_CHUNKS, 128).transpose(2, 0, 1))

    wv2 = np.zeros((128, 2), np.float32)
    wv2[:, 0] = npf(params["comb_head"]["wV"])[:, 0]
    wv2[:, 1] = npf(params["fine_head"]["wV"])[:, 0]
    wv2 = wv2.astype(BF)
    bvc = float(np.asarray(params["comb_head"]["bV"]).reshape(-1)[0])
    bvf = float(np.asarray(params["fine_head"]["bV"]).reshape(-1)[0])

    js = np.asarray(joint_state, np.float32)
    cm = np.asarray(comb_mask).astype(np.float32)
    fm = np.asarray(fine_mask).astype(np.float32)

    in_maps = []
    for c in range(NC):
        bsl = slice(c * BPC, (c + 1) * BPC)
        xt = np.zeros((128, R), np.float32)
        xt[:F] = js[bsl].reshape(R, F).T
        xt = xt.astype(BF)
        cmc, fmc = cm[bsl], fm[bsl]
        mk = np.zeros((1, 4, 128), np.float32)
        mk[0, 0, :64] = np.repeat(cmc, S1)                      # cmpad over (b, s1)
        mk[0, 1, :] = np.repeat(1.0 - cmc, S2)                  # cf over (b, s2)
        qadd = np.zeros((BPC, S2), np.float32)
        for b in range(BPC):
            qadd[b, :S1] += bvc * cmc[b]
            qadd[b] += bvf * (1.0 - cmc[b])
            qadd[b, S1:] += cmc[b] * (-NEG)
            qadd[b] -= (1.0 - fmc[b]) * NEG
        mk[0, 2, :] = qadd.reshape(-1)
        m = {"xt": xt, "bias_all": bias_all, "wv2": wv2, "mk": mk}
        m.update(w)
        in_maps.append(m)
    return in_maps


_NC_CACHE = None


def kernel(joint_state, comb_mask, fine_mask, params):
    global _EXEC, _NC_CACHE
    from concourse._compat import axon_active

    in_maps = _prep_inputs(joint_state, comb_mask, fine_mask, params)
    if not axon_active():
        # native path (real /dev/neuron*): run via NRT directly
        from concourse.bass_utils import run_bass_kernel_spmd
        if _NC_CACHE is None:
            _NC_CACHE = _build()
        res = run_bass_kernel_spmd(_NC_CACHE, in_maps, core_ids=list(range(NC)))
        q = np.stack([r["out"].reshape(-1) for r in res.results])
        return np.ascontiguousarray(q.reshape(B, max(S1, S2)).astype(np.float32))

    if _EXEC is None:
        _EXEC = _make_executor()
    sharded, in_names, out_names, zero_shapes = _EXEC
    concat_in = [np.concatenate([in_maps[c][n] for c in range(NC)], axis=0)
                 for n in in_names]
    concat_zeros = [np.zeros((NC * s[0],) + tuple(s[1:]), d) for (s, d) in zero_shapes]
    out_arrs = sharded(*concat_in, *concat_zeros)
    q = np.asarray(out_arrs[out_names.index("out")])   # [8*1, 128]
    return np.ascontiguousarray(q.reshape(B, max(S1, S2)).astype(np.float32))


# revision 27
# speedup vs baseline: 1.3227x; 1.3227x over previous
"""Trainium2 Bass kernel for nn_CDQNModel (dueling CDQN with residual MLP towers).

Sharding: data-parallel over batch B=16 across 8 NeuronCores (2 batches/core).
Per core the kernel runs the full model on its batch shard:
  - global tower (60->256->512->1024 res blocks + layernorm) over 4096 rows
  - max over s2 -> gf [1024, 64]
  - comb tower (1024->512->256->128) + head -> q_comb [64]
  - fine tower (1084->512->256->128) + head -> q_fine [128]
  - mask/pad combine -> q [2, 64]
Weights are replicated on every core. Activations keep features on SBUF
partitions (layout A); layernorm transposes row-tiles with the PE, computes
stats with bn_stats in fp32, and transposes back. Matmul operands are bf16
(fp32 PSUM accumulation); the LN statistics, bias/beta, and the final mask
combine stay fp32. block2->block3 activations bounce through DRAM to keep
block3's weights resident in SBUF. Emission is software-pipelined so LN
stats windows are covered by independent FC matmuls.
"""

import numpy as np
from contextlib import ExitStack

import jax
import concourse.mybir as mybir
import concourse.tile as tile
from concourse import bacc
from concourse import bass2jax
from jax.sharding import Mesh, PartitionSpec
from jax.experimental.shard_map import shard_map

F32 = mybir.dt.float32
F32R = mybir.dt.bfloat16  # matmul operand dtype (bf16: full PE rate at any N)
F8 = mybir.dt.float8e4    # fp8 for block2/block3 FCs (DoubleRow = 2x PE rate)
DR = mybir.MatmulPerfMode.DoubleRow
SCALE_W = 64.0            # host folds x64 into fp8 weights; ACT relu divides
AF = mybir.ActivationFunctionType
ALU = mybir.AluOpType
AX = mybir.AxisListType

B, S1, S2, F = 16, 32, 64, 60
NC = 8
BPC = B // NC            # batches per core
R = BPC * S1 * S2        # 4096 rows per core
G12 = 512                # rows per group, blocks 1-2
G3 = 512                 # rows per group, block 3
NEG = 100000.0
LN_EPS = 1e-12

# (name, kc_in, u) per res block; kc_in counts 128-wide K chunks (padded)
BLOCKS = {
    "g1": (1, 256), "g2": (2, 512), "g3": (4, 1024),
    "c1": (8, 512), "c2": (4, 256), "c3": (2, 128),
    "f1": (9, 512), "f2": (4, 256), "f3": (2, 128),
}
# bias chunk offsets into the packed [5, 3584] bias tensor
_BOFS = {}
_o = 0
for _n in ["g1", "g2", "g3", "c1", "c2", "c3", "f1", "f2", "f3"]:
    _BOFS[_n] = _o
    _o += BLOCKS[_n][1] // 128
BIAS_CHUNKS = _o  # 28

_EXEC = None  # cached (fn, in_names, n_params, out_shapes)


def _res_fc(nc, tc, name, xks, G, pools, bias_t, wts):
    """FC part of a res block: r1, r2, y = relu-stack + shortcut sum."""
    kc, u = BLOCKS[name]
    mc = u // 128
    P = 128
    bofs = _BOFS[name]
    big, tmp_pool, stat_pool, psum_mm, psum_tr = pools

    f8 = wts["w1"].dtype == F8
    act_dt = F8 if f8 else F32R

    def fc(src_ks, w_t, brow, out, alt0=0, src_t=None):
        n_k = kc if src_t is not None and src_t is _XIN[0] else (
            len(src_ks) if src_ks is not None else mc)
        for m in range(mc):
            ps = psum_mm.tile([P, G], F32, tag="fc")
            b_ap = bias_t[:, brow, bofs + m:bofs + m + 1]
            if f8:
                for kp in range(n_k // 2):
                    nc.tensor.matmul(
                        ps[:, :G],
                        lhsT=w_t[:, 2 * kp:2 * kp + 2, m * 128:(m + 1) * 128],
                        rhs=src_t[:, 2 * kp:2 * kp + 2, :],
                        start=(kp == 0), stop=(kp == n_k // 2 - 1), perf_mode=DR)
                nc.scalar.activation(out[:, m, :], ps[:, :G], AF.Relu,
                                     bias=b_ap, scale=1.0 / SCALE_W)
            else:
                for k in range(n_k):
                    nc.tensor.matmul(ps[:, :G], lhsT=w_t[:, k, m * 128:(m + 1) * 128],
                                     rhs=src_ks[k], start=(k == 0), stop=(k == n_k - 1))
                if (m + alt0) % 2 == 0:
                    nc.scalar.activation(out[:, m, :], ps[:, :G], AF.Relu, bias=b_ap)
                else:
                    nc.vector.tensor_scalar(out[:, m, :], ps[:, :G], b_ap, 0.0,
                                            ALU.add, ALU.max)

    _XIN = [xks if f8 else None]  # fp8: xks is the [128, kc, G] input tile
    xin_t = xks if f8 else None
    xlist = None if f8 else xks
    r1 = big.tile([P, mc, G], act_dt, tag="bigA")
    fc(xlist, wts["w1"], 0, r1, src_t=xin_t)
    r1_ks = None if f8 else [r1[:, m, :] for m in range(mc)]
    r2 = big.tile([P, mc, G], act_dt, tag="bigB")
    fc(r1_ks, wts["w2"], 1, r2, alt0=1, src_t=(r1 if f8 else None))
    r2_ks = None if f8 else [r2[:, m, :] for m in range(mc)]

    y = big.tile([P, mc, G], F32R, tag="bigY")
    fc(xlist, wts["ws"], 3, y, src_t=xin_t)
    for m in range(mc):
        ps = psum_mm.tile([P, G], F32, tag="fc")
        b_ap = bias_t[:, 2, bofs + m:bofs + m + 1]
        t = tmp_pool.tile([P, G], F32R, tag="tmp")
        if f8:
            for kp in range(mc // 2):
                nc.tensor.matmul(
                    ps[:, :G],
                    lhsT=wts["w3"][:, 2 * kp:2 * kp + 2, m * 128:(m + 1) * 128],
                    rhs=r2[:, 2 * kp:2 * kp + 2, :],
                    start=(kp == 0), stop=(kp == mc // 2 - 1), perf_mode=DR)
            nc.scalar.activation(t[:, :G], ps[:, :G], AF.Relu,
                                 bias=b_ap, scale=1.0 / SCALE_W)
        else:
            for k in range(mc):
                nc.tensor.matmul(ps[:, :G], lhsT=wts["w3"][:, k, m * 128:(m + 1) * 128],
                                 rhs=r2_ks[k], start=(k == 0), stop=(k == mc - 1))
            if m % 2 == 0:
                nc.scalar.activation(t[:, :G], ps[:, :G], AF.Relu, bias=b_ap)
            else:
                nc.vector.tensor_scalar(t[:, :G], ps[:, :G], b_ap, 0.0, ALU.add, ALU.max)
        eng = nc.gpsimd if mc >= 4 else nc.vector
        eng.tensor_tensor(y[:, m, :], y[:, m, :], t[:, :G], ALU.add)
    return y


def _res_ln_fwd(nc, tc, name, y, G, pools, rows=None):
    """Forward transpose y -> yB row-tiles, then stats + normalize (DVE)."""
    kc, u = BLOCKS[name]
    mc = u // 128
    P = 128
    big, tmp_pool, stat_pool, psum_mm, psum_tr = pools
    ident = tc._ident
    eps = tc._eps
    act_rows = G if rows is None else rows
    nrt = max(1, act_rows // P)
    rp = min(P, act_rows)
    yB = big.tile([P, nrt, u], F32R, tag="bigYB")
    for rt in range(nrt):
        n_half = (u + 511) // 512
        for h in range(n_half):
            mlo = h * 4
            mhi = min(mc, mlo + 4)
            pst = psum_tr.tile([P, 512], F32R, tag="tr")
            for m in range(mlo, mhi):
                nc.tensor.transpose(pst[:rp, (m - mlo) * 128:(m - mlo + 1) * 128],
                                    y[:, m, rt * rp:(rt + 1) * rp], ident)
            if (rt + h) % 2 == 0:
                nc.scalar.copy(yB[:rp, rt, mlo * 128:mhi * 128],
                               pst[:rp, :(mhi - mlo) * 128])
            else:
                nc.vector.tensor_copy(yB[:rp, rt, mlo * 128:mhi * 128],
                                      pst[:rp, :(mhi - mlo) * 128])
    for rt in range(nrt):
        nseg = max(1, u // 512)
        seg = u // nseg
        sm = stat_pool.tile([P, 16], F32, tag="statsmv")
        stats = sm[:, 0:12].rearrange("p (s x) -> p s x", x=6)
        mv = sm[:, 12:14]
        for s in range(nseg):
            nc.vector.bn_stats(stats[:rp, s, :], yB[:rp, rt, s * seg:(s + 1) * seg])
        nc.vector.bn_aggr(mv[:rp], stats[:rp, 0:nseg, :])
        nc.scalar.activation(mv[:rp, 1:2], mv[:rp, 1:2], AF.Sqrt, bias=eps[:rp])
        nc.vector.reciprocal(mv[:rp, 1:2], mv[:rp, 1:2])
        nc.vector.tensor_scalar(yB[:rp, rt, :], yB[:rp, rt, :], mv[:rp, 0:1],
                                mv[:rp, 1:2], ALU.subtract, ALU.mult)
    return yB


def _res_ln_back(nc, tc, name, yB, G, pools, bias_t, out_gf=None, out_pool=None,
                 rows=None, out_dt=None):
    """Back transpose (+beta) -> yln F32R tiles, or fused s2-max into gf_raw."""
    kc, u = BLOCKS[name]
    mc = u // 128
    P = 128
    bofs = _BOFS[name]
    big, tmp_pool, stat_pool, psum_mm, psum_tr = pools
    ident = tc._ident
    act_rows = G if rows is None else rows
    nrt = max(1, act_rows // P)
    rp = min(P, act_rows)
    yln = None
    if out_gf is None:
        yln = (out_pool or big).tile([P, mc, G], out_dt or F32R, tag=f"yln{name[0]}")
    for m in range(mc):
        pst = psum_tr.tile([P, 512], F32R, tag="tr")
        for rt in range(nrt):
            nc.tensor.transpose(pst[:, rt * rp:(rt + 1) * rp],
                                yB[:rp, rt, m * 128:(m + 1) * 128],
                                ident[:rp, :rp])
        if out_gf is None:
            b_ap = bias_t[:, 4, bofs + m:bofs + m + 1]
            if m % 2 == 0:
                nc.scalar.activation(yln[:, m, :act_rows], pst[:, :act_rows],
                                     AF.Identity, bias=b_ap)
            else:
                nc.vector.tensor_scalar(yln[:, m, :act_rows], pst[:, :act_rows],
                                        b_ap, None, ALU.add)
        else:
            gf_raw, cell0 = out_gf
            ncell = G // 64
            nc.vector.tensor_reduce(
                gf_raw[:, m, cell0:cell0 + ncell],
                pst[:, :G].rearrange("p (c s) -> p c s", s=64),
                axis=AX.X, op=ALU.max)
    return yln


def _build():
    nc = bacc.Bacc("TRN2", target_bir_lowering=False, debug=False)

    xt = nc.dram_tensor("xt", (128, R), F32R, kind="ExternalInput")
    wdr = {}
    for name, (kc, u) in BLOCKS.items():
        wdt = F8 if name in ("g2", "g3") else F32R
        wdr[name] = {
            "w1": nc.dram_tensor(f"{name}_w1", (kc * 128, u), wdt, kind="ExternalInput"),
            "w2": nc.dram_tensor(f"{name}_w2", (u, u), wdt, kind="ExternalInput"),
            "w3": nc.dram_tensor(f"{name}_w3", (u, u), wdt, kind="ExternalInput"),
            "ws": nc.dram_tensor(f"{name}_ws", (kc * 128, u), wdt, kind="ExternalInput"),
        }
    bias_d = nc.dram_tensor("bias_all", (128, 5, BIAS_CHUNKS), F32, kind="ExternalInput")
    wv2_d = nc.dram_tensor("wv2", (128, 2), F32R, kind="ExternalInput")
    mk_d = nc.dram_tensor("mk", (1, 4, 128), F32, kind="ExternalInput")
    h2s = nc.dram_tensor("h2s", (4, 128, R), F8, kind="Internal")
    out_d = nc.dram_tensor("out", (1, BPC * S2), F32, kind="ExternalOutput")

    NGRP = R // G12
    NG3 = R // G3

    with tile.TileContext(nc) as tc, ExitStack() as ctx:
        from concourse.masks import make_identity
        singles = ctx.enter_context(tc.tile_pool(name="singles", bufs=1))
        tmp_pool = ctx.enter_context(tc.tile_pool(name="tmp", bufs=2))
        stat_pool = ctx.enter_context(tc.tile_pool(name="stats", bufs=2))
        psum_mm = ctx.enter_context(tc.tile_pool(name="psum_mm", bufs=5, space="PSUM"))
        psum_tr = ctx.enter_context(tc.tile_pool(name="psum_tr", bufs=3, space="PSUM"))

        ident = singles.tile([128, 128], F32R)
        make_identity(nc, ident)
        tc._ident = ident
        eps = singles.tile([128, 1], F32)
        nc.vector.memset(eps, LN_EPS)
        tc._eps = eps

        bias_t = singles.tile([128, 5, BIAS_CHUNKS], F32)
        nc.sync.dma_start(bias_t, bias_d.ap())
        wv2_t = singles.tile([128, 2], F32R)
        nc.sync.dma_start(wv2_t, wv2_d.ap())
        mk_t = singles.tile([1, 4, 128], F32)
        nc.sync.dma_start(mk_t, mk_d.ap())
        gf_raw = singles.tile([128, 8, 64], F32)
        gf = singles.tile([128, 8, 64], F32R)

        # W3 + h2-read pools opened below the P12 pools (regions don't overlap
        # P12 pools, so their DMAs can run during blocks 1-2)
        mid = ctx.enter_context(ExitStack())
        w3p = mid.enter_context(tc.tile_pool(name="w3p", bufs=1))
        h2rd = mid.enter_context(tc.tile_pool(name="h2rd", bufs=3))

        # ---------------- blocks 1-2 ----------------
        with tc.tile_pool(name="xtp", bufs=1) as xtp, \
             tc.tile_pool(name="w12", bufs=1) as w12, \
             tc.tile_pool(name="big12", bufs=3) as big12, \
             tc.tile_pool(name="yln12", bufs=3) as yln12:
            xT = xtp.tile([128, R], F32R)
            for g in range(NGRP):
                eng = nc.sync if g % 2 == 0 else nc.scalar
                eng.dma_start(xT[:, g * G12:(g + 1) * G12],
                              xt.ap()[:, g * G12:(g + 1) * G12])

            wt12 = {}
            _i = 0
            for name in ("g1", "g2"):
                kc, u = BLOCKS[name]
                wt12[name] = {}
                for wn in ("w1", "w2", "w3", "ws"):
                    d = wdr[name][wn]
                    ckc = d.shape[0] // 128
                    t = w12.tile([128, ckc, u], F8 if name == "g2" else F32R,
                                 tag=f"{name}_{wn}")
                    eng = nc.scalar if _i % 2 == 0 else nc.sync
                    _i += 1
                    eng.dma_start(t, d.ap().rearrange("(c p) f -> p c f", p=128))
                    wt12[name][wn] = t

            # W3 prefetch streams during blocks 1-2 (issued after the loads above)
            wt3 = {}
            for wn in ("w1", "w2", "w3", "ws"):
                d = wdr["g3"][wn]
                ckc = d.shape[0] // 128
                t = w3p.tile([128, ckc, 1024], F8, tag=f"g3_{wn}")
                nc.gpsimd.dma_start(t, d.ap().rearrange("(c p) f -> p c f", p=128))
                wt3[wn] = t

            pools12 = (big12, tmp_pool, stat_pool, psum_mm, psum_tr)
            # software-pipelined emission: back-transposes are emitted after
            # independent FC work so the PE never waits on LN stats
            yB2_prev = None

            def back2_flush(gp):
                y2 = _res_ln_back(nc, tc, "g2", yB2_prev, G12, pools12, bias_t,
                                  out_pool=yln12, out_dt=F8)
                nc.sync.dma_start(
                    h2s.ap()[:, :, gp * G12:(gp + 1) * G12].rearrange("c p j -> p c j"),
                    y2)

            for g in range(NGRP):
                xg = [xT[:, g * G12:(g + 1) * G12]]
                y1 = _res_fc(nc, tc, "g1", xg, G12, pools12, bias_t, wt12["g1"])
                yB1 = _res_ln_fwd(nc, tc, "g1", y1, G12, pools12)
                if g > 0:
                    back2_flush(g - 1)
                yln1 = _res_ln_back(nc, tc, "g1", yB1, G12, pools12, bias_t,
                                    out_pool=yln12, out_dt=F8)
                y2 = _res_fc(nc, tc, "g2", yln1, G12, pools12, bias_t, wt12["g2"])
                yB2_prev = _res_ln_fwd(nc, tc, "g2", y2, G12, pools12)
            back2_flush(NGRP - 1)

        # ---------------- block 3 (+ fused s2-max) ----------------
        with tc.tile_pool(name="big3", bufs=2) as big3:
            pools3 = (big3, tmp_pool, stat_pool, psum_mm, psum_tr)
            yB3_prev = None
            for g in range(NG3):
                h2g = h2rd.tile([128, 4, G3], F8, tag="h2g")
                nc.scalar.dma_start(
                    h2g, h2s.ap()[:, :, g * G3:(g + 1) * G3].rearrange("c p j -> p c j"))
                y3 = _res_fc(nc, tc, "g3", h2g, G3, pools3, bias_t, wt3)
                if g > 0:
                    _res_ln_back(nc, tc, "g3", yB3_prev, G3, pools3, bias_t,
                                 out_gf=(gf_raw, (g - 1) * (G3 // 64)))
                yB3_prev = _res_ln_fwd(nc, tc, "g3", y3, G3, pools3)
            _res_ln_back(nc, tc, "g3", yB3_prev, G3, pools3, bias_t,
                         out_gf=(gf_raw, (NG3 - 1) * (G3 // 64)))
            for m in range(8):
                nc.scalar.activation(gf[:, m, :], gf_raw[:, m, :], AF.Identity,
                                     bias=bias_t[:, 4, _BOFS["g3"] + m:_BOFS["g3"] + m + 1])

        mid.close()  # free W3 + h2rd regions for the tower pools

        # ---------------- comb + fine towers ----------------
        with tc.tile_pool(name="wq", bufs=2) as wq, \
             tc.tile_pool(name="bigq", bufs=2) as bigq:
            def load_w(name):
                kc, u = BLOCKS[name]
                out = {}
                for wn in ("w1", "w2", "w3", "ws"):
                    d = wdr[name][wn]
                    ckc = d.shape[0] // 128
                    # one tag per role sized to the max (9x512) so the pool
                    # holds 2 slots per role, not one per shape
                    tf = wq.tile([128, 9, 512], F32R, tag=f"qw_{wn}")
                    t = tf[:, :ckc, :u]
                    nc.gpsimd.dma_start(t, d.ap().rearrange("(c p) f -> p c f", p=128))
                    out[wn] = t
                return out

            poolsq = (bigq, tmp_pool, stat_pool, psum_mm, psum_tr)
            # fine tower input sfT [1152, 128]
            sfT = bigq.tile([128, 9, 128], F32R, tag="sfT")
            for m in range(8):
                for b in range(BPC):
                    nc.vector.tensor_copy(sfT[:, m, b * 64:(b + 1) * 64],
                                          gf[:, m, b * 32:b * 32 + 1].to_broadcast((128, 64)))
            for b in range(BPC):
                nc.sync.dma_start(sfT[:, 8, b * 64:(b + 1) * 64],
                                  xt.ap()[:, b * (S1 * S2):b * (S1 * S2) + S2])

            # interleave comb (rows=64) and fine (rows=128) towers: each tower's
            # LN stats window is covered by the other tower's FC matmuls
            state = {"c": (gf, 64), "f": (sfT, 128)}
            pending = {}  # tk -> (name, yB)
            for i in (1, 2, 3):
                for tk in ("c", "f"):
                    name = f"{tk}{i}"
                    w_t = load_w(name)
                    kc, u = BLOCKS[name]
                    xcur, rows = state[tk]
                    x_ks = [xcur[:, k, :] for k in range(kc)]
                    yt = _res_fc(nc, tc, name, x_ks, rows, poolsq, bias_t, w_t)
                    yBt = _res_ln_fwd(nc, tc, name, yt, rows, poolsq, rows=rows)
                    pending[tk] = (name, yBt)
                    # flush the OTHER tower's pending back-transpose
                    ok = "f" if tk == "c" else "c"
                    if ok in pending and (i > 1 or tk == "f"):
                        pname, pyB = pending.pop(ok)
                        prows = state[ok][1]
                        ylnp = _res_ln_back(nc, tc, pname, pyB, prows, poolsq,
                                            bias_t, rows=prows)
                        state[ok] = (ylnp, prows)
            for tk in ("c", "f"):
                if tk in pending:
                    pname, pyB = pending.pop(tk)
                    prows = state[tk][1]
                    ylnp = _res_ln_back(nc, tc, pname, pyB, prows, poolsq,
                                        bias_t, rows=prows)
                    state[tk] = (ylnp, prows)
            xc = state["c"][0]
            xf = state["f"][0]
            psh = psum_mm.tile([1, 64], F32, tag="fc")
            nc.tensor.matmul(psh, lhsT=wv2_t[:, 0:1], rhs=xc[:, 0, :], start=True, stop=True)
            qc = bigq.tile([1, 64], F32, tag="qc")
            nc.vector.tensor_copy(qc, psh)
            psf = psum_mm.tile([1, 128], F32, tag="fc")
            nc.tensor.matmul(psf, lhsT=wv2_t[:, 1:2], rhs=xf[:, 0, :], start=True, stop=True)
            qf = bigq.tile([1, 128], F32, tag="qf")
            nc.vector.tensor_copy(qf, psf)

            # combine: q = qf*cf + qadd ; q[:, :, :32] += qc*cmpad
            nc.vector.tensor_tensor(qc, qc, mk_t[:, 0, 0:64], ALU.mult)
            nc.vector.tensor_tensor(qf, qf, mk_t[:, 1, :], ALU.mult)
            q = bigq.tile([1, 128], F32, tag="q")
            nc.vector.tensor_tensor(q, qf, mk_t[:, 2, :], ALU.add)
            qv = q.rearrange("p (b j) -> p b j", j=64)
            qcv = qc.rearrange("p (b j) -> p b j", j=32)
            nc.vector.tensor_tensor(qv[:, :, 0:32], qv[:, :, 0:32], qcv, ALU.add)
            nc.sync.dma_start(out_d.ap(), q)

    nc.finalize()
    return nc


def _make_executor():
    """Build nc once and return a cached jitted shard_map executor
    (mirrors bass2jax.run_bass_via_pjrt's multi-core path)."""
    nc = _build()
    bass2jax.install_neuronx_cc_hook()

    in_names, out_names, out_avals, zero_shapes = [], [], [], []
    for alloc in nc.m.functions[0].allocations:
        if not isinstance(alloc, mybir.MemoryLocationSet):
            continue
        name = alloc.memorylocations[0].name
        if alloc.kind == "ExternalInput":
            in_names.append(name)
        elif alloc.kind == "ExternalOutput":
            out_names.append(name)
            shape = tuple(alloc.tensor_shape)
            dtype = mybir.dt.np(alloc.dtype)
            out_avals.append(jax.core.ShapedArray(shape, dtype))
            zero_shapes.append((shape, dtype))
    partition_name = nc.partition_id_tensor.name if nc.partition_id_tensor else None
    in_names = [n for n in in_names if n != partition_name]
    n_params = len(in_names)
    all_names = in_names + out_names
    if partition_name is not None:
        all_names = all_names + [partition_name]

    def _body(*args):
        operands = list(args)
        if partition_name is not None:
            operands.append(bass2jax.partition_id_tensor())
        outs = bass2jax._bass_exec_p.bind(
            *operands,
            out_avals=tuple(out_avals),
            in_names=tuple(all_names),
            out_names=tuple(out_names),
            lowering_input_output_aliases=(),
            sim_require_finite=True,
            sim_require_nnan=True,
            nc=nc,
        )
        return tuple(outs)

    devices = jax.devices()[:NC]
    mesh = Mesh(np.asarray(devices), ("core",))
    n_outs = len(out_names)
    sharded = jax.jit(
        shard_map(_body, mesh=mesh,
                  in_specs=(PartitionSpec("core"),) * (n_params + n_outs),
                  out_specs=(PartitionSpec("core"),) * n_outs,
                  check_rep=False),
        donate_argnums=tuple(range(n_params, n_params + n_outs)),
        keep_unused=True,
    )
    return sharded, in_names, out_names, zero_shapes


def _prep_inputs(joint_state, comb_mask, fine_mask, params):
    """Host-side: weights (replicated) + per-core shards, keyed by dram name."""
    import ml_dtypes
    BF = ml_dtypes.bfloat16

    def npf(a):
        return np.ascontiguousarray(np.asarray(a, np.float32))

    def npb(a):
        return np.ascontiguousarray(np.asarray(a, np.float32).astype(BF))

    w = {}
    towers = {"g": params["global"], "c": params["comb"], "f": params["fine"]}
    for tk, blocks in towers.items():
        for i, blk in enumerate(blocks):
            name = f"{tk}{i + 1}"
            kc, u = BLOCKS[name]
            w1, b1 = blk["stack"][0]
            w2, b2 = blk["stack"][1]
            w3, b3 = blk["stack"][2]
            ws, bs = blk["ws"], blk["bs"]
            beta = blk["beta"]
            d_in = npf(w1).shape[0]
            w1p = np.zeros((kc * 128, u), np.float32)
            w1p[:d_in] = npf(w1)
            wsp = np.zeros((kc * 128, u), np.float32)
            wsp[:d_in] = npf(ws)
            if name in ("g2", "g3"):
                F8NP = mybir.dt.np(F8)
                w[f"{name}_w1"] = (w1p * SCALE_W).astype(F8NP)
                w[f"{name}_w2"] = (npf(w2) * SCALE_W).astype(F8NP)
                w[f"{name}_w3"] = (npf(w3) * SCALE_W).astype(F8NP)
                w[f"{name}_ws"] = (wsp * SCALE_W).astype(F8NP)
            else:
                w[f"{name}_w1"] = w1p.astype(BF)
                w[f"{name}_w2"] = npb(w2)
                w[f"{name}_w3"] = npb(w3)
                w[f"{name}_ws"] = wsp.astype(BF)
            w[f"{name}_bias"] = np.stack([npf(b1), npf(b2), npf(b3), npf(bs), npf(beta)])

    bias_flat = np.zeros((5, BIAS_CHUNKS * 128), np.float32)
    for name in BLOCKS:
        u = BLOCKS[name][1]
        o = _BOFS[name] * 128
        bias_flat[:, o:o + u] = w.pop(f"{name}_bias")
    # [5, c*128+p] -> [p, 5, c]
    bias_all = np.ascontiguousarray(
        bias_flat.reshape(5, BIAS_CHUNKS, 128).transpose(2, 0, 1))

    wv2 = np.zeros((128, 2), np.float32)
    wv2[:, 0] = npf(params["comb_head"]["wV"])[:, 0]
    wv2[:, 1] = npf(params["fine_head"]["wV"])[:, 0]
    wv2 = wv2.astype(BF)
    bvc = float(np.asarray(params["comb_head"]["bV"]).reshape(-1)[0])
    bvf = float(np.asarray(params["fine_head"]["bV"]).reshape(-1)[0])

    js = np.asarray(joint_state, np.float32)
    cm = np.asarray(comb_mask).astype(np.float32)
    fm = np.asarray(fine_mask).astype(np.float32)

    in_maps = []
    for c in range(NC):
        bsl = slice(c * BPC, (c + 1) * BPC)
        xt = np.zeros((128, R), np.float32)
        xt[:F] = js[bsl].reshape(R, F).T
        xt = xt.astype(BF)
        cmc, fmc = cm[bsl], fm[bsl]
        mk = np.zeros((1, 4, 128), np.float32)
        mk[0, 0, :64] = np.repeat(cmc, S1)                      # cmpad over (b, s1)
        mk[0, 1, :] = np.repeat(1.0 - cmc, S2)                  # cf over (b, s2)
        qadd = np.zeros((BPC, S2), np.float32)
        for b in range(BPC):
            qadd[b, :S1] += bvc * cmc[b]
            qadd[b] += bvf * (1.0 - cmc[b])
            qadd[b, S1:] += cmc[b] * (-NEG)
            qadd[b] -= (1.0 - fmc[b]) * NEG
        mk[0, 2, :] = qadd.reshape(-1)
        m = {"xt": xt, "bias_all": bias_all, "wv2": wv2, "mk": mk}
        m.update(w)
        in_maps.append(m)
    return in_maps


_NC_CACHE = None


def kernel(joint_state, comb_mask, fine_mask, params):
    global _EXEC, _NC_CACHE
    from concourse._compat import axon_active

    in_maps = _prep_inputs(joint_state, comb_mask, fine_mask, params)
    if not axon_active():
        # native path (real /dev/neuron*): run via NRT directly
        from concourse.bass_utils import run_bass_kernel_spmd
        if _NC_CACHE is None:
            _NC_CACHE = _build()
        res = run_bass_kernel_spmd(_NC_CACHE, in_maps, core_ids=list(range(NC)))
        q = np.stack([r["out"].reshape(-1) for r in res.results])
        return np.ascontiguousarray(q.reshape(B, max(S1, S2)).astype(np.float32))

    if _EXEC is None:
        _EXEC = _make_executor()
    sharded, in_names, out_names, zero_shapes = _EXEC
    concat_in = [np.concatenate([in_maps[c][n] for c in range(NC)], axis=0)
                 for n in in_names]
    concat_zeros = [np.zeros((NC * s[0],) + tuple(s[1:]), d) for (s, d) in zero_shapes]
    out_arrs = sharded(*concat_in, *concat_zeros)
    q = np.asarray(out_arrs[out_names.index("out")])   # [8*1, 128]
    return np.ascontiguousarray(q.reshape(B, max(S1, S2)).astype(np.float32))
